# revision 16
# baseline (speedup 1.0000x reference)
"""Bass/Tile TRN2 kernel for nn_DecomposedRotateNet (dense_mlp).

Contract: kernel(**inputs) takes FULL unsharded numpy inputs (as produced by
setup_inputs()) and returns the FULL [4096, 64] float32 output.

Strategy: pure data parallel over 8 NeuronCores — batch 4096 -> 512 rows/core,
small MLP weights replicated. Phase 1 (shift decoder + softmax) runs in bf16,
emitted stage-major across the 4 batch tiles so the per-tile LN chains
pipeline. Phase 2 (the dominant index-net, 137 GFLOP) runs the two big
matmuls in fp8-e4m3 with DoubleRow perf mode (2 fp8 MACs/cell/cycle).
Activations/weights carry fixed power-of-2 scales (S1=SW2=SW3=32) chosen so
all fp8 values sit in e4m3's normal range; the product scale is divided out
exactly in the exp's free affine. Wi3 is zero-padded so both positions of a
pair accumulate into one [128, 512] logits tile (single exp per pair).
Softmax numerators/denominators accumulate into grouped PSUM tiles via
indicator-matmuls, are bulk-evacuated, and divided once at the end with a
single batched reciprocal (replacing 64 expensive per-position DVE
reciprocals).

The build specializes (lru-cached per flag) on the common case where the LN
affines are identity and the linear biases are zero; a fully general path is
kept for other inputs.
"""

import os
import sys
import functools

import numpy as np

for _p in ("/opt/trn_rl_repo",):
    if _p not in sys.path and os.path.isdir(_p):
        sys.path.insert(0, _p)

import concourse.bacc as bacc
import concourse.bass as bass
import concourse.mybir as mybir
import concourse.tile as tile
from concourse import bass_utils
from concourse._compat import with_exitstack
from contextlib import ExitStack

B, BITS, HID = 4096, 64, 512
NCORES = 8
BC = B // NCORES          # 512 batch rows per core
NBT = BC // 128           # 4 batch tiles of 128 (phase 1)
NKC = HID // 128          # 4 chunks of the hidden dim
EPS = 1e-5

F32 = mybir.dt.float32
BF16 = mybir.dt.bfloat16
F8 = mybir.dt.float8e4
MM_DT = BF16              # phase-1 matmul dtype

# fp8 scales (powers of 2; divided out exactly in the exp affine)
S1 = 32.0
SW2 = 32.0
SW3 = 32.0
INV_LG = 1.0 / (S1 * SW2 * SW3)

GT = 32                   # positions per PSUM den/num accumulation group
NGRP = BITS // GT         # 2 groups
GP = GT // 2              # 16 pairs per group

DR = mybir.MatmulPerfMode.DoubleRow

# engine split for phase-2 elementwise ops ("a"=ACT/scalar, "v"=DVE/vector)
H1_ENG = ("v", "v", "v", "v")          # per-fc, both t of a pair
H2_ENG_EVEN = ("a", "a", "a", "a")     # per-kc for even t
H2_ENG_ODD = ("a", "a", "a", "v")      # per-kc for odd t


def _mm(nc, out, lhsT, rhs, start, stop):
    nc.tensor.matmul(out, lhsT, rhs, start=start, stop=stop)


@with_exitstack
def _build_kernel(ctx: ExitStack, tc: "tile.TileContext", io: dict, fast: bool):
    nc = tc.nc
    AF = mybir.ActivationFunctionType
    ALU = mybir.AluOpType

    persist = ctx.enter_context(tc.tile_pool(name="persist", bufs=1))

    def load(name, shape, dt=F32):
        t = persist.tile(shape, dt, name=f"sb_{name}", tag=f"sb_{name}")
        nc.sync.dma_start(t[:], io[name][:])
        return t

    # ---- persistent SBUF tensors (phase-1-critical first) -------------
    X0R = BITS if fast else BITS + 1
    x0a = persist.tile([X0R, BC], MM_DT, name="x0a", tag="x0a")
    nc.sync.dma_start(x0a[0:BITS, :], io["sbT"][:])
    if not fast:
        nc.sync.dma_start(x0a[BITS : BITS + 1, :], io["onesr"][:])
    w1t = load("w1t", [X0R, HID], MM_DT)          # [W1.T (; b1)]
    w2t = [load(f"w2t{i}", [128, HID], MM_DT) for i in range(NKC)]
    w3t = [load(f"w3t{i}", [128, BITS], MM_DT) for i in range(NKC)]
    ident = load("ident", [128, 128])

    if not fast:
        w2b = load("w2b", [1, HID], MM_DT)
        w3b = load("w3b", [1, BITS], MM_DT)
        g1r = load("g1r", [1, HID])
        be1r = load("be1r", [1, HID])
        g2r = load("g2r", [1, HID])
        be2r = load("be2r", [1, HID])
        ones1r = persist.tile([1, BC], MM_DT, name="ones1r", tag="ones1r")
        nc.sync.dma_start(ones1r[:], io["onesr"][:])
        g1bc = persist.tile([128, HID], F32, name="g1bc", tag="g1bc")
        be1bc = persist.tile([128, HID], F32, name="be1bc", tag="be1bc")
        g2bc = persist.tile([128, HID], F32, name="g2bc", tag="g2bc")
        be2bc = persist.tile([128, HID], F32, name="be2bc", tag="be2bc")
        nc.gpsimd.partition_broadcast(g1bc[:], g1r[:])
        nc.gpsimd.partition_broadcast(be1bc[:], be1r[:])
        nc.gpsimd.partition_broadcast(g2bc[:], g2r[:])
        nc.gpsimd.partition_broadcast(be2bc[:], be2r[:])

    wi1bt = load("wi1bt", [BITS, HID], MM_DT)     # (S1*Wi1[:, 64:]).T
    posb = [load(f"posb{i}", [128, BITS]) for i in range(NKC)]
    wi2dr = [
        [load(f"wi2dr{kc}_{i}", [128, 2, 128], F8) for i in range(2)]
        for kc in range(NKC)
    ]
    bi2s = load("bi2s", [128, NKC])               # S1*SW2*bi2 as 4 columns
    # zero-padded Wi3 for pair-stacked logits: wi3p -> rows 0:64 (even t),
    # wi3q -> rows 64:128 (odd t)
    wi3p = [load(f"wi3p{i}", [128, 2, 128], F8) for i in range(2)]
    wi3q = [load(f"wi3q{i}", [128, 2, 128], F8) for i in range(2)]
    bi3c2 = load("bi3c2", [128, 1])               # bi3 stacked twice
    aT2 = load("aT2", [128, BC], BF16)            # a_bits.T stacked twice
    psel = [load(f"psel{g}", [128, GT], MM_DT) for g in range(GP)]

    epsc = persist.tile([128, 1], F32, name="epsc", tag="epsc")
    nc.vector.memset(epsc[:], EPS)

    ssT = persist.tile([BITS, BC], MM_DT, name="ssT", tag="ssT")
    shiftT = [
        persist.tile([128, BC], MM_DT, name=f"shiftT{i}", tag=f"shiftT{i}")
        for i in range(NKC)
    ]
    dennum = persist.tile([BITS, 2 * BC], F32, name="dennum", tag="dennum")

    # =================== phase 1: shift decoder =======================
    # stage-major across the 4 batch tiles; batched [128, NBT] small ops.
    with (
        tc.tile_pool(name="p1s", bufs=4) as p1s,
        tc.tile_pool(name="p1z", bufs=4, space="PSUM") as p1z,
        tc.tile_pool(name="p1t", bufs=2, space="PSUM") as p1t,
    ):
        def ln_stage(zs, gbc, bebc, out_tag):
            """zs: NBT [128, HID] PSUM tiles -> NBT SBUF relu(LN) tiles."""
            mv4 = p1s.tile([128, 2 * NBT], F32, tag=f"mv4_{out_tag}", name="mv4")
            for bt in range(NBT):
                stats = p1s.tile([128, 6], F32, tag=f"st_{out_tag}{bt}", name="st")
                nc.vector.bn_stats(stats[:], zs[bt][:])
                nc.vector.bn_aggr(mv4[:, 2 * bt : 2 * bt + 2], stats[:])
            var4 = mv4[:, 1 : 2 * NBT : 2]
            mean4 = mv4[:, 0 : 2 * NBT : 2]
            std4 = p1s.tile([128, NBT], F32, tag=f"std4_{out_tag}", name="std4")
            nc.scalar.activation(std4[:], var4, AF.Sqrt, bias=epsc[:])
            rinv4 = p1s.tile([128, NBT], F32, tag=f"ri4_{out_tag}", name="rinv4")
            nc.vector.reciprocal(rinv4[:], std4[:])
            nmr4 = p1s.tile([128, NBT], F32, tag=f"nm4_{out_tag}", name="nmr4")
            nc.vector.scalar_tensor_tensor(
                nmr4[:], mean4, -1.0, rinv4[:], op0=ALU.mult, op1=ALU.mult
            )
            outs = []
            for bt in range(NBT):
                if fast:
                    a = p1s.tile([128, HID], F32, tag=f"{out_tag}{bt}", name="a")
                    nc.scalar.activation(
                        a[:], zs[bt][:], AF.Relu,
                        bias=nmr4[:, bt : bt + 1], scale=rinv4[:, bt : bt + 1],
                    )
                else:
                    xn = p1s.tile([128, HID], F32, tag=f"xn_{out_tag}{bt}", name="xn")
                    nc.scalar.activation(
                        xn[:], zs[bt][:], AF.Identity,
                        bias=nmr4[:, bt : bt + 1], scale=rinv4[:, bt : bt + 1],
                    )
                    t1 = p1s.tile([128, HID], F32, tag=f"t1_{out_tag}{bt}", name="t1")
                    nc.vector.tensor_tensor(t1[:], xn[:], gbc[:], op=ALU.mult)
                    t2 = p1s.tile([128, HID], F32, tag=f"t2_{out_tag}{bt}", name="t2")
                    nc.vector.tensor_tensor(t2[:], t1[:], bebc[:], op=ALU.add)
                    a = p1s.tile([128, HID], F32, tag=f"{out_tag}{bt}", name="a")
                    nc.vector.tensor_scalar_max(a[:], t2[:], 0.0)
                outs.append(a)
            return outs

        def transpose_all(srcs, out_tag):
            outs = []
            for bt in range(NBT):
                row = []
                for h in range(NKC):
                    tp = p1t.tile([128, 128], F32, tag="tp", name="tp")
                    nc.tensor.transpose(
                        tp[:], srcs[bt][:, h * 128 : (h + 1) * 128], ident[:]
                    )
                    sb = p1s.tile(
                        [128, 128], MM_DT, tag=f"{out_tag}{bt}_{h}", name="sb"
                    )
                    nc.vector.tensor_copy(sb[:], tp[:])
                    row.append(sb)
                outs.append(row)
            return outs

        z1s = []
        for bt in range(NBT):
            z1 = p1z.tile([128, HID], F32, tag="zz", name="z1")
            _mm(nc, z1[:], x0a[:, bt * 128 : (bt + 1) * 128], w1t[:], True, True)
            z1s.append(z1)
        a1s = ln_stage(z1s, None if fast else g1bc, None if fast else be1bc, "a1")
        a1T = transpose_all(a1s, "a1T")

        z2s = []
        for bt in range(NBT):
            z2 = p1z.tile([128, HID], F32, tag="zz", name="z2")
            for h in range(NKC):
                _mm(nc, z2[:], a1T[bt][h][:], w2t[h][:], h == 0,
                    fast and h == NKC - 1)
            if not fast:
                _mm(nc, z2[:], ones1r[:, bt * 128 : (bt + 1) * 128], w2b[:],
                    False, True)
            z2s.append(z2)
        a2s = ln_stage(z2s, None if fast else g2bc, None if fast else be2bc, "a2")
        a2T = transpose_all(a2s, "a2T")

        # z3 + softmax exp per tile (z3 double-buffered), batched small ops
        ssum4 = p1s.tile([128, NBT], F32, tag="ssum4", name="ssum4")
        ess = []
        for bt in range(NBT):
            z3 = p1t.tile([128, BITS], F32, tag="z3", name="z3")
            for h in range(NKC):
                _mm(nc, z3[:], a2T[bt][h][:], w3t[h][:], h == 0,
                    fast and h == NKC - 1)
            if not fast:
                _mm(nc, z3[:], ones1r[:, bt * 128 : (bt + 1) * 128], w3b[:],
                    False, True)
            es = p1s.tile([128, BITS], F32, tag=f"es{bt}", name="es")
            if fast:
                nc.scalar.activation(
                    es[:], z3[:], AF.Exp, accum_out=ssum4[:, bt : bt + 1]
                )
            else:
                mx = p1s.tile([128, 1], F32, tag=f"mx{bt}", name="mx")
                nc.vector.reduce_max(mx[:], z3[:], axis=mybir.AxisListType.X)
                nmx = p1s.tile([128, 1], F32, tag=f"nmx{bt}", name="nmx")
                nc.vector.tensor_scalar_mul(nmx[:], mx[:], -1.0)
                nc.scalar.activation(
                    es[:], z3[:], AF.Exp, bias=nmx[:],
                    accum_out=ssum4[:, bt : bt + 1],
                )
            ess.append(es)
        rs4 = p1s.tile([128, NBT], F32, tag="rs4", name="rs4")
        nc.vector.reciprocal(rs4[:], ssum4[:])
        for bt in range(NBT):
            ss = p1s.tile([128, BITS], F32, tag=f"ss{bt}", name="ss")
            nc.vector.tensor_scalar_mul(ss[:], ess[bt][:], rs4[:, bt : bt + 1])
            tps = p1t.tile([BITS, 128], F32, tag="tp", name="tps")
            nc.tensor.transpose(tps[:], ss[:], ident[:])
            nc.scalar.copy(ssT[:, bt * 128 : (bt + 1) * 128], tps[:])

        # phase 1.5: shiftT = (S1*Wi1[:,64:]) @ shift_soft.T
        for fc in range(NKC):
            sp = p1z.tile([128, BC], F32, tag="zz", name="sp")
            _mm(nc, sp[:], wi1bt[:, fc * 128 : (fc + 1) * 128], ssT[:], True, True)
            if fc % 2 == 0:
                nc.scalar.copy(shiftT[fc][:], sp[:])
            else:
                nc.vector.tensor_copy(shiftT[fc][:], sp[:])

    # =================== phase 2: index net (fp8 DoubleRow) ============
    with (
        tc.tile_pool(name="p2s", bufs=3) as p2s,
        tc.tile_pool(name="p2e", bufs=3) as p2e,
        tc.tile_pool(name="p2fin", bufs=1) as p2fin,
        tc.tile_pool(name="p2z", bufs=1, space="PSUM") as p2z,
        tc.tile_pool(name="p2lg", bufs=2, space="PSUM") as p2lg,
        tc.tile_pool(name="p2dn", bufs=1, space="PSUM") as p2dn,
    ):
        NP = BITS  # 64 positions
        st = {}
        cur = {"dn": None}

        def h1_build(t):
            tiles = [
                p2s.tile([128, 2, BC], F8, tag=f"h1_{i}", name=f"h1_{i}")
                for i in range(2)
            ]
            for fc in range(NKC):
                j = fc % 2
                dst = tiles[fc // 2][:, j : j + 1, :]
                if H1_ENG[fc] == "v":
                    nc.vector.tensor_scalar(
                        dst, shiftT[fc][:], posb[fc][:, t : t + 1], 0.0,
                        op0=ALU.add, op1=ALU.max,
                    )
                else:
                    nc.scalar.activation(
                        dst, shiftT[fc][:], AF.Relu, bias=posb[fc][:, t : t + 1]
                    )
            st[t] = {"h1": tiles}

        def mm1(t):
            h1 = st[t]["h1"]
            zs = []
            for kc in range(NKC):
                z = p2z.tile([128, BC], F32, tag=f"z{kc}", name="z")
                for i in range(2):
                    nc.tensor.matmul(
                        z[:], wi2dr[kc][i][:], h1[i][:],
                        start=(i == 0), stop=(i == 1), perf_mode=DR,
                    )
                zs.append(z)
            st[t]["z"] = zs

        def h2_build(t):
            zs = st[t]["z"]
            eng = H2_ENG_EVEN if t % 2 == 0 else H2_ENG_ODD
            tiles = [
                p2s.tile([128, 2, BC], F8, tag=f"h2_{i}", name=f"h2_{i}")
                for i in range(2)
            ]
            for kc in range(NKC):
                j = kc % 2
                dst = tiles[kc // 2][:, j : j + 1, :]
                if eng[kc] == "a":
                    nc.scalar.activation(
                        dst, zs[kc][:], AF.Relu, bias=bi2s[:, kc : kc + 1]
                    )
                else:
                    nc.vector.tensor_scalar(
                        dst, zs[kc][:], bi2s[:, kc : kc + 1], 0.0,
                        op0=ALU.add, op1=ALU.max,
                    )
            st[t]["h2"] = tiles
            del st[t]["z"]

        def mm2(t):
            u = t // 2
            if t % 2 == 0:
                lg = p2lg.tile([128, BC], F32, tag="lg", name="lg")
                st[f"lg{u}"] = lg
            lg = st[f"lg{u}"]
            w = wi3p if t % 2 == 0 else wi3q
            h2 = st[t]["h2"]
            for i in range(2):
                nc.tensor.matmul(
                    lg[:], w[i][:], h2[i][:],
                    start=(t % 2 == 0 and i == 0),
                    stop=(t % 2 == 1 and i == 1),
                    perf_mode=DR,
                )

        def eft(u):
            """exp + a-weighting for pair u."""
            lg = st.pop(f"lg{u}")
            e = p2e.tile([128, BC], BF16, tag="e", name="e")
            nc.scalar.activation(e[:], lg[:], AF.Exp, bias=bi3c2[:], scale=INV_LG)
            tmp = p2e.tile([128, BC], BF16, tag="tmp", name="tmp")
            nc.vector.tensor_tensor(tmp[:], e[:], aT2[:], op=ALU.mult)
            st[f"et{u}"] = (e, tmp)

        def cs(u):
            """grouped colsum accumulation + evac for pair u."""
            e, tmp = st.pop(f"et{u}")
            g = u % GP
            if g == 0:
                cur["dn"] = p2dn.tile([GT, 2 * BC], F32, tag="dn", name="dn")
            dn = cur["dn"]
            nc.tensor.matmul(
                dn[:, 0:BC], psel[g][:], e[:], start=(g == 0), stop=(g == GP - 1)
            )
            nc.tensor.matmul(
                dn[:, BC : 2 * BC], psel[g][:], tmp[:],
                start=(g == 0), stop=(g == GP - 1),
            )
            if g == GP - 1:
                grp = u // GP
                nc.vector.tensor_copy(
                    dennum[grp * GT : (grp + 1) * GT, :], dn[:, :]
                )

        # pipelined emission:
        #   h2(t-1) | h1(t) | mm2(t-2) | eft(t//2-2) | cs(t//2-3) | mm1(t)
        for t in range(NP):
            if t >= 1:
                h2_build(t - 1)
            h1_build(t)
            if t >= 2:
                mm2(t - 2)
            if t >= 4 and t % 2 == 0:
                eft((t - 4) // 2)
            if t >= 6 and t % 2 == 0:
                cs((t - 6) // 2)
            mm1(t)
        h2_build(NP - 1)
        mm2(NP - 2)
        eft(NP // 2 - 2)
        cs(NP // 2 - 3)
        mm2(NP - 1)
        eft(NP // 2 - 1)
        cs(NP // 2 - 2)
        cs(NP // 2 - 1)

        # final batched divide: out[t, b] = num/den
        rden = p2fin.tile([BITS, BC], F32, tag="rden", name="rden")
        nc.vector.reciprocal(rden[:], dennum[:, 0:BC])
        outsb = p2fin.tile([BITS, BC], F32, tag="outsb", name="outsb")
        nc.vector.tensor_tensor(outsb[:], dennum[:, BC : 2 * BC], rden[:], op=ALU.mult)
        nc.sync.dma_start(io["out_t"][:], outsb[:])


def _input_specs(fast: bool):
    X0R = BITS if fast else BITS + 1
    specs = [("sbT", [BITS, BC], BF16)]
    specs += [("w1t", [X0R, HID], BF16)]
    specs += [(f"w2t{i}", [128, HID], BF16) for i in range(NKC)]
    specs += [(f"w3t{i}", [128, BITS], BF16) for i in range(NKC)]
    specs += [("ident", [128, 128], F32)]
    if not fast:
        specs += [
            ("w2b", [1, HID], BF16),
            ("w3b", [1, BITS], BF16),
            ("g1r", [1, HID], F32),
            ("be1r", [1, HID], F32),
            ("g2r", [1, HID], F32),
            ("be2r", [1, HID], F32),
            ("onesr", [1, BC], BF16),
        ]
    specs += [("wi1bt", [BITS, HID], BF16)]
    specs += [(f"posb{i}", [128, BITS], F32) for i in range(NKC)]
    specs += [
        (f"wi2dr{kc}_{i}", [128, 2, 128], F8) for kc in range(NKC) for i in range(2)
    ]
    specs += [("bi2s", [128, NKC], F32)]
    specs += [(f"wi3p{i}", [128, 2, 128], F8) for i in range(2)]
    specs += [(f"wi3q{i}", [128, 2, 128], F8) for i in range(2)]
    specs += [("bi3c2", [128, 1], F32)]
    specs += [("aT2", [128, BC], BF16)]
    specs += [(f"psel{g}", [128, GT], BF16) for g in range(GP)]
    return specs


@functools.lru_cache(maxsize=2)
def _get_nc(fast: bool):
    nc = bacc.Bacc("TRN2", target_bir_lowering=False, debug=False, num_devices=NCORES)
    io = {}
    for name, shape, dt in _input_specs(fast):
        io[name] = nc.dram_tensor(name, shape, dt, kind="ExternalInput").ap()
    io["out_t"] = nc.dram_tensor("out_t", [BITS, BC], F32, kind="ExternalOutput").ap()
    with tile.TileContext(nc) as tc:
        _build_kernel(tc, io, fast)
    nc.compile()
    return nc


def _np_dt(dt):
    return mybir.dt.np(dt)


def _q8(x, scale):
    y = np.clip(np.asarray(x, np.float64) * scale, -240.0, 240.0)
    return np.asarray(y, dtype=_np_dt(F8))


def _is_fast(inputs):
    f = lambda n: np.asarray(inputs[n], dtype=np.float32)
    return bool(
        np.all(f("g1") == 1.0) and np.all(f("be1") == 0.0)
        and np.all(f("g2") == 1.0) and np.all(f("be2") == 0.0)
        and np.all(f("b1") == 0.0) and np.all(f("b2") == 0.0)
        and np.all(f("b3") == 0.0)
    )


def _host_prep(inputs, fast):
    f = lambda x: np.ascontiguousarray(np.asarray(x, dtype=np.float32))
    W1, b1 = f(inputs["W1"]), f(inputs["b1"])
    W2, b2 = f(inputs["W2"]), f(inputs["b2"])
    W3, b3 = f(inputs["W3"]), f(inputs["b3"])
    Wi1, bi1 = f(inputs["Wi1"]), f(inputs["bi1"])
    Wi2, bi2 = f(inputs["Wi2"]), f(inputs["bi2"])
    Wi3, bi3 = f(inputs["Wi3"]), f(inputs["bi3"])
    bf = _np_dt(BF16)

    shared = {}
    if fast:
        shared["w1t"] = np.ascontiguousarray(W1.T).astype(bf)
    else:
        shared["w1t"] = np.vstack([W1.T, b1[None, :]]).astype(bf)
        shared["w2b"] = b2[None, :].astype(bf)
        shared["w3b"] = b3[None, :].astype(bf)
        shared["g1r"] = f(inputs["g1"])[None, :]
        shared["be1r"] = f(inputs["be1"])[None, :]
        shared["g2r"] = f(inputs["g2"])[None, :]
        shared["be2r"] = f(inputs["be2"])[None, :]
        shared["onesr"] = np.ones((1, BC), dtype=np.float32).astype(bf)
    w2t_full = W2.T
    for i in range(NKC):
        shared[f"w2t{i}"] = np.ascontiguousarray(
            w2t_full[i * 128 : (i + 1) * 128]
        ).astype(bf)
    w3t_full = W3.T
    for i in range(NKC):
        shared[f"w3t{i}"] = np.ascontiguousarray(
            w3t_full[i * 128 : (i + 1) * 128]
        ).astype(bf)
    shared["wi1bt"] = np.ascontiguousarray(S1 * Wi1[:, BITS:].T).astype(bf)
    posb_full = S1 * (Wi1[:, :BITS] + bi1[:, None])
    for i in range(NKC):
        shared[f"posb{i}"] = np.ascontiguousarray(posb_full[i * 128 : (i + 1) * 128])
    wi2q = np.asarray(_q8(Wi2.T, SW2))
    for kc in range(NKC):
        kcs = slice(kc * 128, (kc + 1) * 128)
        for i in range(2):
            a = wi2q[(2 * i) * 128 : (2 * i + 1) * 128, kcs]
            b = wi2q[(2 * i + 1) * 128 : (2 * i + 2) * 128, kcs]
            shared[f"wi2dr{kc}_{i}"] = np.ascontiguousarray(np.stack([a, b], axis=1))
    shared["bi2s"] = np.ascontiguousarray(
        (S1 * SW2 * bi2).reshape(NKC, 128).T
    ).astype(np.float32)
    wi3full = np.asarray(_q8(Wi3.T, SW3))          # [HID, BITS] fp8
    z64 = np.zeros((128, BITS), dtype=_np_dt(F8))
    for i in range(2):
        a = wi3full[(2 * i) * 128 : (2 * i + 1) * 128, :]
        b = wi3full[(2 * i + 1) * 128 : (2 * i + 2) * 128, :]
        # wi3p: logits land in rows 0:64 (even t); wi3q: rows 64:128 (odd t)
        shared[f"wi3p{i}"] = np.ascontiguousarray(
            np.stack([np.concatenate([a, z64], 1), np.concatenate([b, z64], 1)], 1)
        )
        shared[f"wi3q{i}"] = np.ascontiguousarray(
            np.stack([np.concatenate([z64, a], 1), np.concatenate([z64, b], 1)], 1)
        )
    shared["bi3c2"] = np.concatenate([bi3, bi3])[:, None].astype(np.float32)
    for g in range(GP):
        m = np.zeros((128, GT), np.float32)
        m[0:BITS, 2 * g] = 1.0
        m[BITS:128, 2 * g + 1] = 1.0
        shared[f"psel{g}"] = m.astype(bf)
    shared["ident"] = np.eye(128, dtype=np.float32)
    return shared


def _make_in_maps(inputs, fast):
    shared = _host_prep(inputs, fast)
    bf = _np_dt(BF16)
    a_bits = np.asarray(inputs["a_bits"], dtype=np.float32)
    shift_bits = np.asarray(inputs["shift_bits"], dtype=np.float32)
    in_maps = []
    for c in range(NCORES):
        rows = slice(c * BC, (c + 1) * BC)
        m = dict(shared)
        m["sbT"] = np.ascontiguousarray(shift_bits[rows].T).astype(bf)
        aT = np.ascontiguousarray(a_bits[rows].T)
        m["aT2"] = np.vstack([aT, aT]).astype(bf)
        in_maps.append(m)
    return in_maps


def run_on_cores(inputs, trace=False):
    """Returns (full_output [4096, 64] f32, BassKernelResults)."""
    fast = _is_fast(inputs)
    nc = _get_nc(fast)
    in_maps = _make_in_maps(inputs, fast)
    res = bass_utils.run_bass_kernel_spmd(
        nc, in_maps, list(range(NCORES)), trace=trace
    )
    out = np.empty((B, BITS), dtype=np.float32)
    for c in range(NCORES):
        out[c * BC : (c + 1) * BC] = res.results[c]["out_t"].T
    return out, res


def kernel(**inputs) -> np.ndarray:
    out, _ = run_on_cores(inputs, trace=False)
    return out


# revision 17
# speedup vs baseline: 1.1725x; 1.1725x over previous
"""Bass/Tile TRN2 kernel for nn_DecomposedRotateNet (dense_mlp).

Contract: kernel(**inputs) takes FULL unsharded numpy inputs (as produced by
setup_inputs()) and returns the FULL [4096, 64] float32 output.

Strategy: pure data parallel over 8 NeuronCores — batch 4096 -> 512 rows/core,
small MLP weights replicated. Phase 1 (shift decoder + softmax) runs in bf16,
emitted stage-major across the 4 batch tiles so the per-tile LN chains
pipeline. Phase 2 (the dominant index-net, 137 GFLOP) runs the two big
matmuls in fp8-e4m3 with DoubleRow perf mode (2 fp8 MACs/cell/cycle).
Activations/weights carry fixed power-of-2 scales (S1=SW2=SW3=32) chosen so
all fp8 values sit in e4m3's normal range; the product scale is divided out
exactly in the exp's free affine. Wi3 is zero-padded so both positions of a
pair accumulate into one [128, 512] logits tile (single exp per pair).
Softmax numerators/denominators accumulate into grouped PSUM tiles via
indicator-matmuls, are bulk-evacuated, and divided once at the end with a
single batched reciprocal (replacing 64 expensive per-position DVE
reciprocals).

The build specializes (lru-cached per flag) on the common case where the LN
affines are identity and the linear biases are zero; a fully general path is
kept for other inputs.
"""

import os
import sys
import functools

import numpy as np

for _p in ("/opt/trn_rl_repo",):
    if _p not in sys.path and os.path.isdir(_p):
        sys.path.insert(0, _p)

import concourse.bacc as bacc
import concourse.bass as bass
import concourse.mybir as mybir
import concourse.tile as tile
from concourse import bass_utils
from concourse._compat import with_exitstack
from contextlib import ExitStack

B, BITS, HID = 4096, 64, 512
NCORES = 8
BC = B // NCORES          # 512 batch rows per core
NBT = BC // 128           # 4 batch tiles of 128 (phase 1)
NKC = HID // 128          # 4 chunks of the hidden dim
EPS = 1e-5

F32 = mybir.dt.float32
BF16 = mybir.dt.bfloat16
F8 = mybir.dt.float8e4
MM_DT = BF16              # phase-1 matmul dtype

# fp8 scales (powers of 2; divided out exactly in the exp affine)
S1 = 32.0
SW2 = 32.0
SW3 = 32.0
INV_LG = 1.0 / (S1 * SW2 * SW3)

GT = 32                   # positions per PSUM den/num accumulation group
NGRP = BITS // GT         # 2 groups
GP = GT // 2              # 16 pairs per group

DR = mybir.MatmulPerfMode.DoubleRow

# engine split for phase-2 elementwise ops ("a"=ACT/scalar, "v"=DVE/vector)
H1_ENG = ("v", "v", "v", "v")          # per-fc, both t of a pair
H2_ENG_EVEN = ("a", "a", "a", "a")     # per-kc for even t
H2_ENG_ODD = ("a", "a", "a", "v")      # per-kc for odd t


def _mm(nc, out, lhsT, rhs, start, stop):
    nc.tensor.matmul(out, lhsT, rhs, start=start, stop=stop)


@with_exitstack
def _build_kernel(ctx: ExitStack, tc: "tile.TileContext", io: dict, fast: bool):
    nc = tc.nc
    AF = mybir.ActivationFunctionType
    ALU = mybir.AluOpType

    persist = ctx.enter_context(tc.tile_pool(name="persist", bufs=1))

    def load(name, shape, dt=F32):
        t = persist.tile(shape, dt, name=f"sb_{name}", tag=f"sb_{name}")
        nc.sync.dma_start(t[:], io[name][:])
        return t

    # ---- persistent SBUF tensors (phase-1-critical first) -------------
    X0R = BITS if fast else BITS + 1
    x0a = persist.tile([X0R, BC], MM_DT, name="x0a", tag="x0a")
    nc.sync.dma_start(x0a[0:BITS, :], io["sbT"][:])
    if not fast:
        nc.sync.dma_start(x0a[BITS : BITS + 1, :], io["onesr"][:])
    w1t = load("w1t", [X0R, HID], MM_DT)          # [W1.T (; b1)]
    w2t = [load(f"w2t{i}", [128, HID], MM_DT) for i in range(NKC)]
    w3t = [load(f"w3t{i}", [128, BITS], MM_DT) for i in range(NKC)]
    ident = load("ident", [128, 128])

    if not fast:
        w2b = load("w2b", [1, HID], MM_DT)
        w3b = load("w3b", [1, BITS], MM_DT)
        g1r = load("g1r", [1, HID])
        be1r = load("be1r", [1, HID])
        g2r = load("g2r", [1, HID])
        be2r = load("be2r", [1, HID])
        ones1r = persist.tile([1, BC], MM_DT, name="ones1r", tag="ones1r")
        nc.sync.dma_start(ones1r[:], io["onesr"][:])
        g1bc = persist.tile([128, HID], F32, name="g1bc", tag="g1bc")
        be1bc = persist.tile([128, HID], F32, name="be1bc", tag="be1bc")
        g2bc = persist.tile([128, HID], F32, name="g2bc", tag="g2bc")
        be2bc = persist.tile([128, HID], F32, name="be2bc", tag="be2bc")
        nc.gpsimd.partition_broadcast(g1bc[:], g1r[:])
        nc.gpsimd.partition_broadcast(be1bc[:], be1r[:])
        nc.gpsimd.partition_broadcast(g2bc[:], g2r[:])
        nc.gpsimd.partition_broadcast(be2bc[:], be2r[:])

    wi1bt = load("wi1bt", [BITS, HID], MM_DT)     # (S1*Wi1[:, 64:]).T
    posb = [load(f"posb{i}", [128, BITS]) for i in range(NKC)]
    wi2dr = [
        [load(f"wi2dr{kc}_{i}", [128, 2, 128], F8) for i in range(2)]
        for kc in range(NKC)
    ]
    bi2s = load("bi2s", [128, NKC])               # S1*SW2*bi2 as 4 columns
    # zero-padded Wi3 for pair-stacked logits: wi3p -> rows 0:64 (even t),
    # wi3q -> rows 64:128 (odd t)
    wi3p = [load(f"wi3p{i}", [128, 2, 128], F8) for i in range(2)]
    wi3q = [load(f"wi3q{i}", [128, 2, 128], F8) for i in range(2)]
    bi3c2 = load("bi3c2", [128, 1])               # bi3 stacked twice
    aT2 = load("aT2", [128, BC], BF16)            # a_bits.T stacked twice
    psel = [load(f"psel{g}", [128, GT], MM_DT) for g in range(GP)]

    epsc = persist.tile([128, 1], F32, name="epsc", tag="epsc")
    nc.vector.memset(epsc[:], EPS)

    ssT = persist.tile([BITS, BC], MM_DT, name="ssT", tag="ssT")
    shiftT = [
        persist.tile([128, BC], MM_DT, name=f"shiftT{i}", tag=f"shiftT{i}")
        for i in range(NKC)
    ]
    dennum = persist.tile([BITS, 2 * BC], F32, name="dennum", tag="dennum")

    # =================== phase 1: shift decoder =======================
    # stage-major across the 4 batch tiles; batched [128, NBT] small ops.
    with (
        tc.tile_pool(name="p1s", bufs=4) as p1s,
        tc.tile_pool(name="p1z", bufs=4, space="PSUM") as p1z,
        tc.tile_pool(name="p1t", bufs=2, space="PSUM") as p1t,
    ):
        def ln_stage(zs, gbc, bebc, out_tag):
            """zs: NBT [128, HID] PSUM tiles -> NBT SBUF relu(LN) tiles."""
            mv4 = p1s.tile([128, 2 * NBT], F32, tag=f"mv4_{out_tag}", name="mv4")
            for bt in range(NBT):
                stats = p1s.tile([128, 6], F32, tag=f"st_{out_tag}{bt}", name="st")
                nc.vector.bn_stats(stats[:], zs[bt][:])
                nc.vector.bn_aggr(mv4[:, 2 * bt : 2 * bt + 2], stats[:])
            var4 = mv4[:, 1 : 2 * NBT : 2]
            mean4 = mv4[:, 0 : 2 * NBT : 2]
            std4 = p1s.tile([128, NBT], F32, tag=f"std4_{out_tag}", name="std4")
            nc.scalar.activation(std4[:], var4, AF.Sqrt, bias=epsc[:])
            rinv4 = p1s.tile([128, NBT], F32, tag=f"ri4_{out_tag}", name="rinv4")
            nc.vector.reciprocal(rinv4[:], std4[:])
            nmr4 = p1s.tile([128, NBT], F32, tag=f"nm4_{out_tag}", name="nmr4")
            nc.vector.scalar_tensor_tensor(
                nmr4[:], mean4, -1.0, rinv4[:], op0=ALU.mult, op1=ALU.mult
            )
            outs = []
            for bt in range(NBT):
                if fast:
                    a = p1s.tile([128, HID], F32, tag=f"{out_tag}{bt}", name="a")
                    nc.scalar.activation(
                        a[:], zs[bt][:], AF.Relu,
                        bias=nmr4[:, bt : bt + 1], scale=rinv4[:, bt : bt + 1],
                    )
                else:
                    xn = p1s.tile([128, HID], F32, tag=f"xn_{out_tag}{bt}", name="xn")
                    nc.scalar.activation(
                        xn[:], zs[bt][:], AF.Identity,
                        bias=nmr4[:, bt : bt + 1], scale=rinv4[:, bt : bt + 1],
                    )
                    t1 = p1s.tile([128, HID], F32, tag=f"t1_{out_tag}{bt}", name="t1")
                    nc.vector.tensor_tensor(t1[:], xn[:], gbc[:], op=ALU.mult)
                    t2 = p1s.tile([128, HID], F32, tag=f"t2_{out_tag}{bt}", name="t2")
                    nc.vector.tensor_tensor(t2[:], t1[:], bebc[:], op=ALU.add)
                    a = p1s.tile([128, HID], F32, tag=f"{out_tag}{bt}", name="a")
                    nc.vector.tensor_scalar_max(a[:], t2[:], 0.0)
                outs.append(a)
            return outs

        def transpose_all(srcs, out_tag):
            outs = []
            for bt in range(NBT):
                row = []
                for h in range(NKC):
                    tp = p1t.tile([128, 128], F32, tag="tp", name="tp")
                    nc.tensor.transpose(
                        tp[:], srcs[bt][:, h * 128 : (h + 1) * 128], ident[:]
                    )
                    sb = p1s.tile(
                        [128, 128], MM_DT, tag=f"{out_tag}{bt}_{h}", name="sb"
                    )
                    nc.scalar.copy(sb[:], tp[:])
                    row.append(sb)
                outs.append(row)
            return outs

        z1s = []
        for bt in range(NBT):
            z1 = p1z.tile([128, HID], F32, tag="zz", name="z1")
            _mm(nc, z1[:], x0a[:, bt * 128 : (bt + 1) * 128], w1t[:], True, True)
            z1s.append(z1)
        a1s = ln_stage(z1s, None if fast else g1bc, None if fast else be1bc, "a1")
        a1T = transpose_all(a1s, "a1T")

        z2s = []
        for bt in range(NBT):
            z2 = p1z.tile([128, HID], F32, tag="zz", name="z2")
            for h in range(NKC):
                _mm(nc, z2[:], a1T[bt][h][:], w2t[h][:], h == 0,
                    fast and h == NKC - 1)
            if not fast:
                _mm(nc, z2[:], ones1r[:, bt * 128 : (bt + 1) * 128], w2b[:],
                    False, True)
            z2s.append(z2)
        a2s = ln_stage(z2s, None if fast else g2bc, None if fast else be2bc, "a2")
        a2T = transpose_all(a2s, "a2T")

        # z3 + softmax exp per tile (z3 double-buffered), batched small ops
        ssum4 = p1s.tile([128, NBT], F32, tag="ssum4", name="ssum4")
        ess = []
        for bt in range(NBT):
            z3 = p1t.tile([128, BITS], F32, tag="z3", name="z3")
            for h in range(NKC):
                _mm(nc, z3[:], a2T[bt][h][:], w3t[h][:], h == 0,
                    fast and h == NKC - 1)
            if not fast:
                _mm(nc, z3[:], ones1r[:, bt * 128 : (bt + 1) * 128], w3b[:],
                    False, True)
            mx = p1s.tile([128, 1], F32, tag=f"mx{bt}", name="mx")
            nc.vector.reduce_max(mx[:], z3[:], axis=mybir.AxisListType.X)
            nmx = p1s.tile([128, 1], F32, tag=f"nmx{bt}", name="nmx")
            nc.vector.tensor_scalar_mul(nmx[:], mx[:], -1.0)
            es = p1s.tile([128, BITS], F32, tag=f"es{bt}", name="es")
            nc.scalar.activation(
                es[:], z3[:], AF.Exp, bias=nmx[:],
                accum_out=ssum4[:, bt : bt + 1],
            )
            ess.append(es)
        rs4 = p1s.tile([128, NBT], F32, tag="rs4", name="rs4")
        nc.vector.reciprocal(rs4[:], ssum4[:])
        for bt in range(NBT):
            ss = p1s.tile([128, BITS], F32, tag=f"ss{bt}", name="ss")
            nc.vector.tensor_scalar_mul(ss[:], ess[bt][:], rs4[:, bt : bt + 1])
            tps = p1t.tile([BITS, 128], F32, tag="tp", name="tps")
            nc.tensor.transpose(tps[:], ss[:], ident[:])
            nc.scalar.copy(ssT[:, bt * 128 : (bt + 1) * 128], tps[:])

        # phase 1.5: shiftT = (S1*Wi1[:,64:]) @ shift_soft.T
        for fc in range(NKC):
            sp = p1z.tile([128, BC], F32, tag="zz", name="sp")
            _mm(nc, sp[:], wi1bt[:, fc * 128 : (fc + 1) * 128], ssT[:], True, True)
            nc.scalar.copy(shiftT[fc][:], sp[:])

    # =================== phase 2: index net (fp8 DoubleRow) ============
    with (
        tc.tile_pool(name="p2s", bufs=3) as p2s,
        tc.tile_pool(name="p2e", bufs=3) as p2e,
        tc.tile_pool(name="p2fin", bufs=1) as p2fin,
        tc.tile_pool(name="p2z", bufs=1, space="PSUM") as p2z,
        tc.tile_pool(name="p2lg", bufs=2, space="PSUM") as p2lg,
        tc.tile_pool(name="p2dn", bufs=1, space="PSUM") as p2dn,
    ):
        NP = BITS  # 64 positions
        st = {}
        cur = {"dn": None}

        def h1_build(t):
            tiles = [
                p2s.tile([128, 2, BC], F8, tag=f"h1_{i}", name=f"h1_{i}")
                for i in range(2)
            ]
            for fc in range(NKC):
                j = fc % 2
                dst = tiles[fc // 2][:, j : j + 1, :]
                if H1_ENG[fc] == "v":
                    nc.vector.tensor_scalar(
                        dst, shiftT[fc][:], posb[fc][:, t : t + 1], 0.0,
                        op0=ALU.add, op1=ALU.max,
                    )
                else:
                    nc.scalar.activation(
                        dst, shiftT[fc][:], AF.Relu, bias=posb[fc][:, t : t + 1]
                    )
            st[t] = {"h1": tiles}

        def mm1(t):
            h1 = st[t]["h1"]
            zs = []
            for kc in range(NKC):
                z = p2z.tile([128, BC], F32, tag=f"z{kc}", name="z")
                for i in range(2):
                    nc.tensor.matmul(
                        z[:], wi2dr[kc][i][:], h1[i][:],
                        start=(i == 0), stop=(i == 1), perf_mode=DR,
                    )
                zs.append(z)
            st[t]["z"] = zs

        def h2_build(t):
            zs = st[t]["z"]
            eng = H2_ENG_EVEN if t % 2 == 0 else H2_ENG_ODD
            tiles = [
                p2s.tile([128, 2, BC], F8, tag=f"h2_{i}", name=f"h2_{i}")
                for i in range(2)
            ]
            for kc in range(NKC):
                j = kc % 2
                dst = tiles[kc // 2][:, j : j + 1, :]
                if eng[kc] == "a":
                    nc.scalar.activation(
                        dst, zs[kc][:], AF.Relu, bias=bi2s[:, kc : kc + 1]
                    )
                else:
                    nc.vector.tensor_scalar(
                        dst, zs[kc][:], bi2s[:, kc : kc + 1], 0.0,
                        op0=ALU.add, op1=ALU.max,
                    )
            st[t]["h2"] = tiles
            del st[t]["z"]

        def mm2(t):
            u = t // 2
            if t % 2 == 0:
                lg = p2lg.tile([128, BC], F32, tag="lg", name="lg")
                st[f"lg{u}"] = lg
            lg = st[f"lg{u}"]
            w = wi3p if t % 2 == 0 else wi3q
            h2 = st[t]["h2"]
            for i in range(2):
                nc.tensor.matmul(
                    lg[:], w[i][:], h2[i][:],
                    start=(t % 2 == 0 and i == 0),
                    stop=(t % 2 == 1 and i == 1),
                    perf_mode=DR,
                )

        def eft(u):
            """exp + a-weighting for pair u."""
            lg = st.pop(f"lg{u}")
            e = p2e.tile([128, BC], BF16, tag="e", name="e")
            nc.scalar.activation(e[:], lg[:], AF.Exp, bias=bi3c2[:], scale=INV_LG)
            tmp = p2e.tile([128, BC], BF16, tag="tmp", name="tmp")
            nc.vector.tensor_tensor(tmp[:], e[:], aT2[:], op=ALU.mult)
            st[f"et{u}"] = (e, tmp)

        def cs(u):
            """grouped colsum accumulation + evac for pair u."""
            e, tmp = st.pop(f"et{u}")
            g = u % GP
            if g == 0:
                cur["dn"] = p2dn.tile([GT, 2 * BC], F32, tag="dn", name="dn")
            dn = cur["dn"]
            nc.tensor.matmul(
                dn[:, 0:BC], psel[g][:], e[:], start=(g == 0), stop=(g == GP - 1)
            )
            nc.tensor.matmul(
                dn[:, BC : 2 * BC], psel[g][:], tmp[:],
                start=(g == 0), stop=(g == GP - 1),
            )
            if g == GP - 1:
                grp = u // GP
                nc.vector.tensor_copy(
                    dennum[grp * GT : (grp + 1) * GT, :], dn[:, :]
                )

        # pipelined emission:
        #   h2(t-1) | h1(t) | mm2(t-2) | eft(t//2-2) | cs(t//2-3) | mm1(t)
        for t in range(NP):
            if t >= 1:
                h2_build(t - 1)
            h1_build(t)
            if t >= 2:
                mm2(t - 2)
            if t >= 4 and t % 2 == 0:
                eft((t - 4) // 2)
            if t >= 6 and t % 2 == 0:
                cs((t - 6) // 2)
            mm1(t)
        h2_build(NP - 1)
        mm2(NP - 2)
        eft(NP // 2 - 2)
        cs(NP // 2 - 3)
        mm2(NP - 1)
        eft(NP // 2 - 1)
        cs(NP // 2 - 2)
        cs(NP // 2 - 1)

        # final batched divide: out[t, b] = num/den
        rden = p2fin.tile([BITS, BC], F32, tag="rden", name="rden")
        nc.vector.reciprocal(rden[:], dennum[:, 0:BC])
        outsb = p2fin.tile([BITS, BC], F32, tag="outsb", name="outsb")
        nc.vector.tensor_tensor(outsb[:], dennum[:, BC : 2 * BC], rden[:], op=ALU.mult)
        nc.sync.dma_start(io["out_t"][:], outsb[:])


def _input_specs(fast: bool):
    X0R = BITS if fast else BITS + 1
    specs = [("sbT", [BITS, BC], BF16)]
    specs += [("w1t", [X0R, HID], BF16)]
    specs += [(f"w2t{i}", [128, HID], BF16) for i in range(NKC)]
    specs += [(f"w3t{i}", [128, BITS], BF16) for i in range(NKC)]
    specs += [("ident", [128, 128], F32)]
    if not fast:
        specs += [
            ("w2b", [1, HID], BF16),
            ("w3b", [1, BITS], BF16),
            ("g1r", [1, HID], F32),
            ("be1r", [1, HID], F32),
            ("g2r", [1, HID], F32),
            ("be2r", [1, HID], F32),
            ("onesr", [1, BC], BF16),
        ]
    specs += [("wi1bt", [BITS, HID], BF16)]
    specs += [(f"posb{i}", [128, BITS], F32) for i in range(NKC)]
    specs += [
        (f"wi2dr{kc}_{i}", [128, 2, 128], F8) for kc in range(NKC) for i in range(2)
    ]
    specs += [("bi2s", [128, NKC], F32)]
    specs += [(f"wi3p{i}", [128, 2, 128], F8) for i in range(2)]
    specs += [(f"wi3q{i}", [128, 2, 128], F8) for i in range(2)]
    specs += [("bi3c2", [128, 1], F32)]
    specs += [("aT2", [128, BC], BF16)]
    specs += [(f"psel{g}", [128, GT], BF16) for g in range(GP)]
    return specs


@functools.lru_cache(maxsize=2)
def _get_nc(fast: bool):
    nc = bacc.Bacc("TRN2", target_bir_lowering=False, debug=False, num_devices=NCORES)
    io = {}
    for name, shape, dt in _input_specs(fast):
        io[name] = nc.dram_tensor(name, shape, dt, kind="ExternalInput").ap()
    io["out_t"] = nc.dram_tensor("out_t", [BITS, BC], F32, kind="ExternalOutput").ap()
    with tile.TileContext(nc) as tc:
        _build_kernel(tc, io, fast)
    nc.compile()
    return nc


def _np_dt(dt):
    return mybir.dt.np(dt)


def _q8(x, scale):
    y = np.clip(np.asarray(x, np.float64) * scale, -240.0, 240.0)
    return np.asarray(y, dtype=_np_dt(F8))


def _is_fast(inputs):
    f = lambda n: np.asarray(inputs[n], dtype=np.float32)
    return bool(
        np.all(f("g1") == 1.0) and np.all(f("be1") == 0.0)
        and np.all(f("g2") == 1.0) and np.all(f("be2") == 0.0)
        and np.all(f("b1") == 0.0) and np.all(f("b2") == 0.0)
        and np.all(f("b3") == 0.0)
    )


def _host_prep(inputs, fast):
    f = lambda x: np.ascontiguousarray(np.asarray(x, dtype=np.float32))
    W1, b1 = f(inputs["W1"]), f(inputs["b1"])
    W2, b2 = f(inputs["W2"]), f(inputs["b2"])
    W3, b3 = f(inputs["W3"]), f(inputs["b3"])
    Wi1, bi1 = f(inputs["Wi1"]), f(inputs["bi1"])
    Wi2, bi2 = f(inputs["Wi2"]), f(inputs["bi2"])
    Wi3, bi3 = f(inputs["Wi3"]), f(inputs["bi3"])
    bf = _np_dt(BF16)

    shared = {}
    if fast:
        shared["w1t"] = np.ascontiguousarray(W1.T).astype(bf)
    else:
        shared["w1t"] = np.vstack([W1.T, b1[None, :]]).astype(bf)
        shared["w2b"] = b2[None, :].astype(bf)
        shared["w3b"] = b3[None, :].astype(bf)
        shared["g1r"] = f(inputs["g1"])[None, :]
        shared["be1r"] = f(inputs["be1"])[None, :]
        shared["g2r"] = f(inputs["g2"])[None, :]
        shared["be2r"] = f(inputs["be2"])[None, :]
        shared["onesr"] = np.ones((1, BC), dtype=np.float32).astype(bf)
    w2t_full = W2.T
    for i in range(NKC):
        shared[f"w2t{i}"] = np.ascontiguousarray(
            w2t_full[i * 128 : (i + 1) * 128]
        ).astype(bf)
    w3t_full = W3.T
    for i in range(NKC):
        shared[f"w3t{i}"] = np.ascontiguousarray(
            w3t_full[i * 128 : (i + 1) * 128]
        ).astype(bf)
    shared["wi1bt"] = np.ascontiguousarray(S1 * Wi1[:, BITS:].T).astype(bf)
    posb_full = S1 * (Wi1[:, :BITS] + bi1[:, None])
    for i in range(NKC):
        shared[f"posb{i}"] = np.ascontiguousarray(posb_full[i * 128 : (i + 1) * 128])
    wi2q = np.asarray(_q8(Wi2.T, SW2))
    for kc in range(NKC):
        kcs = slice(kc * 128, (kc + 1) * 128)
        for i in range(2):
            a = wi2q[(2 * i) * 128 : (2 * i + 1) * 128, kcs]
            b = wi2q[(2 * i + 1) * 128 : (2 * i + 2) * 128, kcs]
            shared[f"wi2dr{kc}_{i}"] = np.ascontiguousarray(np.stack([a, b], axis=1))
    shared["bi2s"] = np.ascontiguousarray(
        (S1 * SW2 * bi2).reshape(NKC, 128).T
    ).astype(np.float32)
    wi3full = np.asarray(_q8(Wi3.T, SW3))          # [HID, BITS] fp8
    z64 = np.zeros((128, BITS), dtype=_np_dt(F8))
    for i in range(2):
        a = wi3full[(2 * i) * 128 : (2 * i + 1) * 128, :]
        b = wi3full[(2 * i + 1) * 128 : (2 * i + 2) * 128, :]
        # wi3p: logits land in rows 0:64 (even t); wi3q: rows 64:128 (odd t)
        shared[f"wi3p{i}"] = np.ascontiguousarray(
            np.stack([np.concatenate([a, z64], 1), np.concatenate([b, z64], 1)], 1)
        )
        shared[f"wi3q{i}"] = np.ascontiguousarray(
            np.stack([np.concatenate([z64, a], 1), np.concatenate([z64, b], 1)], 1)
        )
    shared["bi3c2"] = np.concatenate([bi3, bi3])[:, None].astype(np.float32)
    for g in range(GP):
        m = np.zeros((128, GT), np.float32)
        m[0:BITS, 2 * g] = 1.0
        m[BITS:128, 2 * g + 1] = 1.0
        shared[f"psel{g}"] = m.astype(bf)
    shared["ident"] = np.eye(128, dtype=np.float32)
    return shared


def _make_in_maps(inputs, fast):
    shared = _host_prep(inputs, fast)
    bf = _np_dt(BF16)
    a_bits = np.asarray(inputs["a_bits"], dtype=np.float32)
    shift_bits = np.asarray(inputs["shift_bits"], dtype=np.float32)
    in_maps = []
    for c in range(NCORES):
        rows = slice(c * BC, (c + 1) * BC)
        m = dict(shared)
        m["sbT"] = np.ascontiguousarray(shift_bits[rows].T).astype(bf)
        aT = np.ascontiguousarray(a_bits[rows].T)
        m["aT2"] = np.vstack([aT, aT]).astype(bf)
        in_maps.append(m)
    return in_maps


def run_on_cores(inputs, trace=False):
    """Returns (full_output [4096, 64] f32, BassKernelResults)."""
    fast = _is_fast(inputs)
    nc = _get_nc(fast)
    in_maps = _make_in_maps(inputs, fast)
    res = bass_utils.run_bass_kernel_spmd(
        nc, in_maps, list(range(NCORES)), trace=trace
    )
    out = np.empty((B, BITS), dtype=np.float32)
    for c in range(NCORES):
        out[c * BC : (c + 1) * BC] = res.results[c]["out_t"].T
    return out, res


def kernel(**inputs) -> np.ndarray:
    out, _ = run_on_cores(inputs, trace=False)
    return out


# revision 18
# speedup vs baseline: 1.1727x; 1.0002x over previous
"""Bass/Tile TRN2 kernel for nn_DecomposedRotateNet (dense_mlp).

Contract: kernel(**inputs) takes FULL unsharded numpy inputs (as produced by
setup_inputs()) and returns the FULL [4096, 64] float32 output.

Strategy: pure data parallel over 8 NeuronCores — batch 4096 -> 512 rows/core,
small MLP weights replicated. Phase 1 (shift decoder + softmax) runs in bf16,
emitted stage-major across the 4 batch tiles so the per-tile LN chains
pipeline. Phase 2 (the dominant index-net, 137 GFLOP) runs the two big
matmuls in fp8-e4m3 with DoubleRow perf mode (2 fp8 MACs/cell/cycle).
Activations/weights carry fixed power-of-2 scales (S1=SW2=SW3=32) chosen so
all fp8 values sit in e4m3's normal range; the product scale is divided out
exactly in the exp's free affine. Wi3 is zero-padded so both positions of a
pair accumulate into one [128, 512] logits tile (single exp per pair).
Softmax numerators/denominators accumulate into grouped PSUM tiles via
indicator-matmuls, are bulk-evacuated, and divided once at the end with a
single batched reciprocal (replacing 64 expensive per-position DVE
reciprocals).

The build specializes (lru-cached per flag) on the common case where the LN
affines are identity and the linear biases are zero; a fully general path is
kept for other inputs.
"""

import os
import sys
import functools

import numpy as np

for _p in ("/opt/trn_rl_repo",):
    if _p not in sys.path and os.path.isdir(_p):
        sys.path.insert(0, _p)

import concourse.bacc as bacc
import concourse.bass as bass
import concourse.mybir as mybir
import concourse.tile as tile
from concourse import bass_utils
from concourse._compat import with_exitstack
from contextlib import ExitStack

B, BITS, HID = 4096, 64, 512
NCORES = 8
BC = B // NCORES          # 512 batch rows per core
NBT = BC // 128           # 4 batch tiles of 128 (phase 1)
NKC = HID // 128          # 4 chunks of the hidden dim
EPS = 1e-5

F32 = mybir.dt.float32
BF16 = mybir.dt.bfloat16
F8 = mybir.dt.float8e4
MM_DT = BF16              # phase-1 matmul dtype

# fp8 scales (powers of 2; divided out exactly in the exp affine)
S1 = 32.0
SW2 = 32.0
SW3 = 32.0
INV_LG = 1.0 / (S1 * SW2 * SW3)

GT = 32                   # positions per PSUM den/num accumulation group
NGRP = BITS // GT         # 2 groups
GP = GT // 2              # 16 pairs per group

DR = mybir.MatmulPerfMode.DoubleRow

# engine split for phase-2 elementwise ops ("a"=ACT/scalar, "v"=DVE/vector)
H1_ENG = ("v", "v", "v", "v")          # per-fc, both t of a pair
H2_ENG_EVEN = ("a", "a", "a", "a")     # per-kc for even t
H2_ENG_ODD = ("a", "a", "a", "v")      # per-kc for odd t


def _mm(nc, out, lhsT, rhs, start, stop):
    nc.tensor.matmul(out, lhsT, rhs, start=start, stop=stop)


@with_exitstack
def _build_kernel(ctx: ExitStack, tc: "tile.TileContext", io: dict, fast: bool):
    nc = tc.nc
    AF = mybir.ActivationFunctionType
    ALU = mybir.AluOpType

    persist = ctx.enter_context(tc.tile_pool(name="persist", bufs=1))

    def load(name, shape, dt=F32):
        t = persist.tile(shape, dt, name=f"sb_{name}", tag=f"sb_{name}")
        nc.sync.dma_start(t[:], io[name][:])
        return t

    # ---- persistent SBUF tensors (phase-1-critical first) -------------
    X0R = BITS if fast else BITS + 1
    x0a = persist.tile([X0R, BC], MM_DT, name="x0a", tag="x0a")
    nc.sync.dma_start(x0a[0:BITS, :], io["sbT"][:])
    if not fast:
        nc.sync.dma_start(x0a[BITS : BITS + 1, :], io["onesr"][:])
    w1t = load("w1t", [X0R, HID], MM_DT)          # [W1.T (; b1)]
    w2t = [load(f"w2t{i}", [128, HID], MM_DT) for i in range(NKC)]
    w3t = [load(f"w3t{i}", [128, BITS], MM_DT) for i in range(NKC)]
    ident = load("ident", [128, 128])

    if not fast:
        w2b = load("w2b", [1, HID], MM_DT)
        w3b = load("w3b", [1, BITS], MM_DT)
        g1r = load("g1r", [1, HID])
        be1r = load("be1r", [1, HID])
        g2r = load("g2r", [1, HID])
        be2r = load("be2r", [1, HID])
        ones1r = persist.tile([1, BC], MM_DT, name="ones1r", tag="ones1r")
        nc.sync.dma_start(ones1r[:], io["onesr"][:])
        g1bc = persist.tile([128, HID], F32, name="g1bc", tag="g1bc")
        be1bc = persist.tile([128, HID], F32, name="be1bc", tag="be1bc")
        g2bc = persist.tile([128, HID], F32, name="g2bc", tag="g2bc")
        be2bc = persist.tile([128, HID], F32, name="be2bc", tag="be2bc")
        nc.gpsimd.partition_broadcast(g1bc[:], g1r[:])
        nc.gpsimd.partition_broadcast(be1bc[:], be1r[:])
        nc.gpsimd.partition_broadcast(g2bc[:], g2r[:])
        nc.gpsimd.partition_broadcast(be2bc[:], be2r[:])

    wi1bt = load("wi1bt", [BITS, HID], MM_DT)     # (S1*Wi1[:, 64:]).T
    posb = [load(f"posb{i}", [128, BITS]) for i in range(NKC)]
    wi2dr = [
        [load(f"wi2dr{kc}_{i}", [128, 2, 128], F8) for i in range(2)]
        for kc in range(NKC)
    ]
    bi2s = load("bi2s", [128, NKC])               # S1*SW2*bi2 as 4 columns
    # zero-padded Wi3 for pair-stacked logits: wi3p -> rows 0:64 (even t),
    # wi3q -> rows 64:128 (odd t)
    wi3p = [load(f"wi3p{i}", [128, 2, 128], F8) for i in range(2)]
    wi3q = [load(f"wi3q{i}", [128, 2, 128], F8) for i in range(2)]
    bi3c2 = load("bi3c2", [128, 1])               # bi3 stacked twice
    aT2 = load("aT2", [128, BC], BF16)            # a_bits.T stacked twice
    psel = [load(f"psel{g}", [128, GT], MM_DT) for g in range(GP)]

    epsc = persist.tile([128, 1], F32, name="epsc", tag="epsc")
    nc.vector.memset(epsc[:], EPS)

    ssT = persist.tile([BITS, BC], MM_DT, name="ssT", tag="ssT")
    shiftT = [
        persist.tile([128, BC], MM_DT, name=f"shiftT{i}", tag=f"shiftT{i}")
        for i in range(NKC)
    ]
    dennum = persist.tile([BITS, 2 * BC], F32, name="dennum", tag="dennum")

    # =================== phase 1: shift decoder =======================
    # stage-major across the 4 batch tiles; batched [128, NBT] small ops.
    with (
        tc.tile_pool(name="p1s", bufs=4) as p1s,
        tc.tile_pool(name="p1z", bufs=4, space="PSUM") as p1z,
        tc.tile_pool(name="p1t", bufs=2, space="PSUM") as p1t,
    ):
        def ln_stage(zs, gbc, bebc, out_tag):
            """zs: NBT [128, HID] PSUM tiles -> NBT SBUF relu(LN) tiles."""
            mv4 = p1s.tile([128, 2 * NBT], F32, tag=f"mv4_{out_tag}", name="mv4")
            for bt in range(NBT):
                stats = p1s.tile([128, 6], F32, tag=f"st_{out_tag}{bt}", name="st")
                nc.vector.bn_stats(stats[:], zs[bt][:])
                nc.vector.bn_aggr(mv4[:, 2 * bt : 2 * bt + 2], stats[:])
            var4 = mv4[:, 1 : 2 * NBT : 2]
            mean4 = mv4[:, 0 : 2 * NBT : 2]
            std4 = p1s.tile([128, NBT], F32, tag=f"std4_{out_tag}", name="std4")
            nc.scalar.activation(std4[:], var4, AF.Sqrt, bias=epsc[:])
            rinv4 = p1s.tile([128, NBT], F32, tag=f"ri4_{out_tag}", name="rinv4")
            nc.vector.reciprocal(rinv4[:], std4[:])
            nmr4 = p1s.tile([128, NBT], F32, tag=f"nm4_{out_tag}", name="nmr4")
            nc.vector.scalar_tensor_tensor(
                nmr4[:], mean4, -1.0, rinv4[:], op0=ALU.mult, op1=ALU.mult
            )
            outs = []
            for bt in range(NBT):
                if fast:
                    a = p1s.tile([128, HID], F32, tag=f"{out_tag}{bt}", name="a")
                    nc.scalar.activation(
                        a[:], zs[bt][:], AF.Relu,
                        bias=nmr4[:, bt : bt + 1], scale=rinv4[:, bt : bt + 1],
                    )
                else:
                    xn = p1s.tile([128, HID], F32, tag=f"xn_{out_tag}{bt}", name="xn")
                    nc.scalar.activation(
                        xn[:], zs[bt][:], AF.Identity,
                        bias=nmr4[:, bt : bt + 1], scale=rinv4[:, bt : bt + 1],
                    )
                    t1 = p1s.tile([128, HID], F32, tag=f"t1_{out_tag}{bt}", name="t1")
                    nc.vector.tensor_tensor(t1[:], xn[:], gbc[:], op=ALU.mult)
                    t2 = p1s.tile([128, HID], F32, tag=f"t2_{out_tag}{bt}", name="t2")
                    nc.vector.tensor_tensor(t2[:], t1[:], bebc[:], op=ALU.add)
                    a = p1s.tile([128, HID], F32, tag=f"{out_tag}{bt}", name="a")
                    nc.vector.tensor_scalar_max(a[:], t2[:], 0.0)
                outs.append(a)
            return outs

        def transpose_all(srcs, out_tag):
            outs = []
            for bt in range(NBT):
                row = []
                for h in range(NKC):
                    tp = p1t.tile([128, 128], F32, tag="tp", name="tp")
                    nc.tensor.transpose(
                        tp[:], srcs[bt][:, h * 128 : (h + 1) * 128], ident[:]
                    )
                    sb = p1s.tile(
                        [128, 128], MM_DT, tag=f"{out_tag}{bt}_{h}", name="sb"
                    )
                    nc.scalar.copy(sb[:], tp[:])
                    row.append(sb)
                outs.append(row)
            return outs

        z1s = []
        for bt in range(NBT):
            z1 = p1z.tile([128, HID], F32, tag="zz", name="z1")
            _mm(nc, z1[:], x0a[:, bt * 128 : (bt + 1) * 128], w1t[:], True, True)
            z1s.append(z1)
        a1s = ln_stage(z1s, None if fast else g1bc, None if fast else be1bc, "a1")
        a1T = transpose_all(a1s, "a1T")

        z2s = []
        for bt in range(NBT):
            z2 = p1z.tile([128, HID], F32, tag="zz", name="z2")
            for h in range(NKC):
                _mm(nc, z2[:], a1T[bt][h][:], w2t[h][:], h == 0,
                    fast and h == NKC - 1)
            if not fast:
                _mm(nc, z2[:], ones1r[:, bt * 128 : (bt + 1) * 128], w2b[:],
                    False, True)
            z2s.append(z2)
        a2s = ln_stage(z2s, None if fast else g2bc, None if fast else be2bc, "a2")
        a2T = transpose_all(a2s, "a2T")

        # z3 + softmax exp per tile (z3 double-buffered), batched small ops
        ssum4 = p1s.tile([128, NBT], F32, tag="ssum4", name="ssum4")
        ess = []
        for bt in range(NBT):
            z3 = p1t.tile([128, BITS], F32, tag="z3", name="z3")
            for h in range(NKC):
                _mm(nc, z3[:], a2T[bt][h][:], w3t[h][:], h == 0,
                    fast and h == NKC - 1)
            if not fast:
                _mm(nc, z3[:], ones1r[:, bt * 128 : (bt + 1) * 128], w3b[:],
                    False, True)
            mx = p1s.tile([128, 1], F32, tag=f"mx{bt}", name="mx")
            nc.vector.reduce_max(mx[:], z3[:], axis=mybir.AxisListType.X)
            nmx = p1s.tile([128, 1], F32, tag=f"nmx{bt}", name="nmx")
            nc.vector.tensor_scalar_mul(nmx[:], mx[:], -1.0)
            es = p1s.tile([128, BITS], F32, tag=f"es{bt}", name="es")
            nc.scalar.activation(
                es[:], z3[:], AF.Exp, bias=nmx[:],
                accum_out=ssum4[:, bt : bt + 1],
            )
            ess.append(es)
        rs4 = p1s.tile([128, NBT], F32, tag="rs4", name="rs4")
        nc.vector.reciprocal(rs4[:], ssum4[:])
        for bt in range(NBT):
            ss = p1s.tile([128, BITS], F32, tag=f"ss{bt}", name="ss")
            nc.vector.tensor_scalar_mul(ss[:], ess[bt][:], rs4[:, bt : bt + 1])
            tps = p1t.tile([BITS, 128], F32, tag="tp", name="tps")
            nc.tensor.transpose(tps[:], ss[:], ident[:])
            nc.scalar.copy(ssT[:, bt * 128 : (bt + 1) * 128], tps[:])

        # phase 1.5: shiftT = (S1*Wi1[:,64:]) @ shift_soft.T
        for fc in range(NKC):
            sp = p1z.tile([128, BC], F32, tag="zz", name="sp")
            _mm(nc, sp[:], wi1bt[:, fc * 128 : (fc + 1) * 128], ssT[:], True, True)
            nc.scalar.copy(shiftT[fc][:], sp[:])

    # =================== phase 2: index net (fp8 DoubleRow) ============
    with (
        tc.tile_pool(name="p2s", bufs=3) as p2s,
        tc.tile_pool(name="p2e", bufs=3) as p2e,
        tc.tile_pool(name="p2fin", bufs=1) as p2fin,
        tc.tile_pool(name="p2z", bufs=1, space="PSUM") as p2z,
        tc.tile_pool(name="p2lg", bufs=2, space="PSUM") as p2lg,
        tc.tile_pool(name="p2dn", bufs=1, space="PSUM") as p2dn,
    ):
        NP = BITS  # 64 positions
        st = {}
        cur = {"dn": None}

        def h1_build(t):
            tiles = [
                p2s.tile([128, 2, BC], F8, tag=f"h1_{i}", name=f"h1_{i}")
                for i in range(2)
            ]
            for fc in range(NKC):
                j = fc % 2
                dst = tiles[fc // 2][:, j : j + 1, :]
                if H1_ENG[fc] == "v":
                    nc.vector.tensor_scalar(
                        dst, shiftT[fc][:], posb[fc][:, t : t + 1], 0.0,
                        op0=ALU.add, op1=ALU.max,
                    )
                else:
                    nc.scalar.activation(
                        dst, shiftT[fc][:], AF.Relu, bias=posb[fc][:, t : t + 1]
                    )
            st[t] = {"h1": tiles}

        def mm1(t):
            h1 = st[t]["h1"]
            zs = []
            for kc in range(NKC):
                z = p2z.tile(
                    [128, BC], F32, tag=f"z{kc}", name="z",
                    bufs=2 if kc == 0 else 1,
                )
                for i in range(2):
                    nc.tensor.matmul(
                        z[:], wi2dr[kc][i][:], h1[i][:],
                        start=(i == 0), stop=(i == 1), perf_mode=DR,
                    )
                zs.append(z)
            st[t]["z"] = zs

        def h2_build(t):
            zs = st[t]["z"]
            eng = H2_ENG_EVEN if t % 2 == 0 else H2_ENG_ODD
            tiles = [
                p2s.tile([128, 2, BC], F8, tag=f"h2_{i}", name=f"h2_{i}")
                for i in range(2)
            ]
            for kc in range(NKC):
                j = kc % 2
                dst = tiles[kc // 2][:, j : j + 1, :]
                if eng[kc] == "a":
                    nc.scalar.activation(
                        dst, zs[kc][:], AF.Relu, bias=bi2s[:, kc : kc + 1]
                    )
                else:
                    nc.vector.tensor_scalar(
                        dst, zs[kc][:], bi2s[:, kc : kc + 1], 0.0,
                        op0=ALU.add, op1=ALU.max,
                    )
            st[t]["h2"] = tiles
            del st[t]["z"]

        def mm2(t):
            u = t // 2
            if t % 2 == 0:
                lg = p2lg.tile([128, BC], F32, tag="lg", name="lg")
                st[f"lg{u}"] = lg
            lg = st[f"lg{u}"]
            w = wi3p if t % 2 == 0 else wi3q
            h2 = st[t]["h2"]
            for i in range(2):
                nc.tensor.matmul(
                    lg[:], w[i][:], h2[i][:],
                    start=(t % 2 == 0 and i == 0),
                    stop=(t % 2 == 1 and i == 1),
                    perf_mode=DR,
                )

        def eft(u):
            """exp + a-weighting for pair u."""
            lg = st.pop(f"lg{u}")
            e = p2e.tile([128, BC], BF16, tag="e", name="e")
            nc.scalar.activation(e[:], lg[:], AF.Exp, bias=bi3c2[:], scale=INV_LG)
            tmp = p2e.tile([128, BC], BF16, tag="tmp", name="tmp")
            nc.vector.tensor_tensor(tmp[:], e[:], aT2[:], op=ALU.mult)
            st[f"et{u}"] = (e, tmp)

        def cs(u):
            """grouped colsum accumulation + evac for pair u."""
            e, tmp = st.pop(f"et{u}")
            g = u % GP
            if g == 0:
                cur["dn"] = p2dn.tile([2 * GT, BC], F32, tag="dn", name="dn")
            dn = cur["dn"]
            nc.tensor.matmul(
                dn[0:GT, :], psel[g][:], e[:], start=(g == 0), stop=(g == GP - 1)
            )
            nc.tensor.matmul(
                dn[GT : 2 * GT, :], psel[g][:], tmp[:],
                start=(g == 0), stop=(g == GP - 1),
            )
            if g == GP - 1:
                grp = u // GP
                nc.vector.tensor_copy(
                    dennum[grp * GT : (grp + 1) * GT, 0:BC], dn[0:GT, :]
                )
                nc.vector.tensor_copy(
                    dennum[grp * GT : (grp + 1) * GT, BC : 2 * BC],
                    dn[GT : 2 * GT, :],
                )

        # pipelined emission:
        #   h2(t-1) | h1(t) | mm2(t-2) | eft(t//2-2) | cs(t//2-3) | mm1(t)
        for t in range(NP):
            if t >= 1:
                h2_build(t - 1)
            h1_build(t)
            if t >= 2:
                mm2(t - 2)
            if t >= 4 and t % 2 == 0:
                eft((t - 4) // 2)
            if t >= 6 and t % 2 == 0:
                cs((t - 6) // 2)
            mm1(t)
        h2_build(NP - 1)
        mm2(NP - 2)
        eft(NP // 2 - 2)
        cs(NP // 2 - 3)
        mm2(NP - 1)
        eft(NP // 2 - 1)
        cs(NP // 2 - 2)
        cs(NP // 2 - 1)

        # final batched divide: out[t, b] = num/den
        rden = p2fin.tile([BITS, BC], F32, tag="rden", name="rden")
        nc.vector.reciprocal_approx_fast(rden[:], dennum[:, 0:BC])
        outsb = p2fin.tile([BITS, BC], F32, tag="outsb", name="outsb")
        nc.vector.tensor_tensor(outsb[:], dennum[:, BC : 2 * BC], rden[:], op=ALU.mult)
        nc.sync.dma_start(io["out_t"][:], outsb[:])


def _input_specs(fast: bool):
    X0R = BITS if fast else BITS + 1
    specs = [("sbT", [BITS, BC], BF16)]
    specs += [("w1t", [X0R, HID], BF16)]
    specs += [(f"w2t{i}", [128, HID], BF16) for i in range(NKC)]
    specs += [(f"w3t{i}", [128, BITS], BF16) for i in range(NKC)]
    specs += [("ident", [128, 128], F32)]
    if not fast:
        specs += [
            ("w2b", [1, HID], BF16),
            ("w3b", [1, BITS], BF16),
            ("g1r", [1, HID], F32),
            ("be1r", [1, HID], F32),
            ("g2r", [1, HID], F32),
            ("be2r", [1, HID], F32),
            ("onesr", [1, BC], BF16),
        ]
    specs += [("wi1bt", [BITS, HID], BF16)]
    specs += [(f"posb{i}", [128, BITS], F32) for i in range(NKC)]
    specs += [
        (f"wi2dr{kc}_{i}", [128, 2, 128], F8) for kc in range(NKC) for i in range(2)
    ]
    specs += [("bi2s", [128, NKC], F32)]
    specs += [(f"wi3p{i}", [128, 2, 128], F8) for i in range(2)]
    specs += [(f"wi3q{i}", [128, 2, 128], F8) for i in range(2)]
    specs += [("bi3c2", [128, 1], F32)]
    specs += [("aT2", [128, BC], BF16)]
    specs += [(f"psel{g}", [128, GT], BF16) for g in range(GP)]
    return specs


@functools.lru_cache(maxsize=2)
def _get_nc(fast: bool):
    nc = bacc.Bacc("TRN2", target_bir_lowering=False, debug=False, num_devices=NCORES)
    io = {}
    for name, shape, dt in _input_specs(fast):
        io[name] = nc.dram_tensor(name, shape, dt, kind="ExternalInput").ap()
    io["out_t"] = nc.dram_tensor("out_t", [BITS, BC], F32, kind="ExternalOutput").ap()
    with tile.TileContext(nc) as tc:
        _build_kernel(tc, io, fast)
    nc.compile()
    return nc


def _np_dt(dt):
    return mybir.dt.np(dt)


def _q8(x, scale):
    y = np.clip(np.asarray(x, np.float64) * scale, -240.0, 240.0)
    return np.asarray(y, dtype=_np_dt(F8))


def _is_fast(inputs):
    f = lambda n: np.asarray(inputs[n], dtype=np.float32)
    return bool(
        np.all(f("g1") == 1.0) and np.all(f("be1") == 0.0)
        and np.all(f("g2") == 1.0) and np.all(f("be2") == 0.0)
        and np.all(f("b1") == 0.0) and np.all(f("b2") == 0.0)
        and np.all(f("b3") == 0.0)
    )


def _host_prep(inputs, fast):
    f = lambda x: np.ascontiguousarray(np.asarray(x, dtype=np.float32))
    W1, b1 = f(inputs["W1"]), f(inputs["b1"])
    W2, b2 = f(inputs["W2"]), f(inputs["b2"])
    W3, b3 = f(inputs["W3"]), f(inputs["b3"])
    Wi1, bi1 = f(inputs["Wi1"]), f(inputs["bi1"])
    Wi2, bi2 = f(inputs["Wi2"]), f(inputs["bi2"])
    Wi3, bi3 = f(inputs["Wi3"]), f(inputs["bi3"])
    bf = _np_dt(BF16)

    shared = {}
    if fast:
        shared["w1t"] = np.ascontiguousarray(W1.T).astype(bf)
    else:
        shared["w1t"] = np.vstack([W1.T, b1[None, :]]).astype(bf)
        shared["w2b"] = b2[None, :].astype(bf)
        shared["w3b"] = b3[None, :].astype(bf)
        shared["g1r"] = f(inputs["g1"])[None, :]
        shared["be1r"] = f(inputs["be1"])[None, :]
        shared["g2r"] = f(inputs["g2"])[None, :]
        shared["be2r"] = f(inputs["be2"])[None, :]
        shared["onesr"] = np.ones((1, BC), dtype=np.float32).astype(bf)
    w2t_full = W2.T
    for i in range(NKC):
        shared[f"w2t{i}"] = np.ascontiguousarray(
            w2t_full[i * 128 : (i + 1) * 128]
        ).astype(bf)
    w3t_full = W3.T
    for i in range(NKC):
        shared[f"w3t{i}"] = np.ascontiguousarray(
            w3t_full[i * 128 : (i + 1) * 128]
        ).astype(bf)
    shared["wi1bt"] = np.ascontiguousarray(S1 * Wi1[:, BITS:].T).astype(bf)
    posb_full = S1 * (Wi1[:, :BITS] + bi1[:, None])
    for i in range(NKC):
        shared[f"posb{i}"] = np.ascontiguousarray(posb_full[i * 128 : (i + 1) * 128])
    wi2q = np.asarray(_q8(Wi2.T, SW2))
    for kc in range(NKC):
        kcs = slice(kc * 128, (kc + 1) * 128)
        for i in range(2):
            a = wi2q[(2 * i) * 128 : (2 * i + 1) * 128, kcs]
            b = wi2q[(2 * i + 1) * 128 : (2 * i + 2) * 128, kcs]
            shared[f"wi2dr{kc}_{i}"] = np.ascontiguousarray(np.stack([a, b], axis=1))
    shared["bi2s"] = np.ascontiguousarray(
        (S1 * SW2 * bi2).reshape(NKC, 128).T
    ).astype(np.float32)
    wi3full = np.asarray(_q8(Wi3.T, SW3))          # [HID, BITS] fp8
    z64 = np.zeros((128, BITS), dtype=_np_dt(F8))
    for i in range(2):
        a = wi3full[(2 * i) * 128 : (2 * i + 1) * 128, :]
        b = wi3full[(2 * i + 1) * 128 : (2 * i + 2) * 128, :]
        # wi3p: logits land in rows 0:64 (even t); wi3q: rows 64:128 (odd t)
        shared[f"wi3p{i}"] = np.ascontiguousarray(
            np.stack([np.concatenate([a, z64], 1), np.concatenate([b, z64], 1)], 1)
        )
        shared[f"wi3q{i}"] = np.ascontiguousarray(
            np.stack([np.concatenate([z64, a], 1), np.concatenate([z64, b], 1)], 1)
        )
    shared["bi3c2"] = np.concatenate([bi3, bi3])[:, None].astype(np.float32)
    for g in range(GP):
        m = np.zeros((128, GT), np.float32)
        m[0:BITS, 2 * g] = 1.0
        m[BITS:128, 2 * g + 1] = 1.0
        shared[f"psel{g}"] = m.astype(bf)
    shared["ident"] = np.eye(128, dtype=np.float32)
    return shared


def _make_in_maps(inputs, fast):
    shared = _host_prep(inputs, fast)
    bf = _np_dt(BF16)
    a_bits = np.asarray(inputs["a_bits"], dtype=np.float32)
    shift_bits = np.asarray(inputs["shift_bits"], dtype=np.float32)
    in_maps = []
    for c in range(NCORES):
        rows = slice(c * BC, (c + 1) * BC)
        m = dict(shared)
        m["sbT"] = np.ascontiguousarray(shift_bits[rows].T).astype(bf)
        aT = np.ascontiguousarray(a_bits[rows].T)
        m["aT2"] = np.vstack([aT, aT]).astype(bf)
        in_maps.append(m)
    return in_maps


def run_on_cores(inputs, trace=False):
    """Returns (full_output [4096, 64] f32, BassKernelResults)."""
    fast = _is_fast(inputs)
    nc = _get_nc(fast)
    in_maps = _make_in_maps(inputs, fast)
    res = bass_utils.run_bass_kernel_spmd(
        nc, in_maps, list(range(NCORES)), trace=trace
    )
    out = np.empty((B, BITS), dtype=np.float32)
    for c in range(NCORES):
        out[c * BC : (c + 1) * BC] = res.results[c]["out_t"].T
    return out, res


def kernel(**inputs) -> np.ndarray:
    out, _ = run_on_cores(inputs, trace=False)
    return out
